# revision 10
# baseline (speedup 1.0000x reference)
"""TLGv4 block-sparse self-attention on 8 trn2 NeuronCores.

Sharding: tensor-parallel over the 8 KV groups (1 group = 4 Q heads + 1 K +
1 V head per core). Each core computes its group's QKV projection columns,
RoPE, block-sparse attention for its 4 Q heads, and a row-sharded partial of
the dense output projection (f16). Host sums the 8 partials (+ b_dense).

Single fused pipeline: QKV slice matmuls, attention pairs and dense units
are interleaved in one tensor-engine stream so the PE never drains and the
ACT engine (exp) works in the shadow of matmuls throughout:
  - qkvT[c, t] = wq_g @ hidden^T accumulated in 3 PSUM banks per 512-token
    slice; bias-add (DVE) + RoPE (DVE) produce qS2/kT2 with the query/key
    rows replicated to partitions 64:127 (SBUF->SBUF DMA) so score matmuls
    for TWO k-chunks run concurrently as PE row-group tiles (K=64 each).
  - v^T via XBAR transpose-DMA; 64 ones-columns per chunk make the PV
    matmul (M=128) emit softmax denominators as ctx rows 64:127.
  - exp on ACT over a whole score duo [128,1024] (2 PSUM banks) in one
    ACTIVATE; block-sparsity masks (memset + causal tri multiply) on GpSimd.
  - 1/Z via DVE reciprocal_approx_fast direct from PSUM; 4 normalize muls
    write ctx_sb in the dense lhsT layout.
  - dense: 4 units per 128-token pair, PSUM -> f16 staging (copies split
    ACT/DVE) -> one 512KB store per pair.
"""
import numpy as np
from contextlib import ExitStack

import concourse.bacc as bacc
import concourse.bass as bass
import concourse.mybir as mybir
import concourse.tile as tile
from concourse.bass_utils import run_bass_kernel_spmd

F32 = mybir.dt.float32
F16 = mybir.dt.float16
AF = mybir.ActivationFunctionType

S = 2048
HID = 2048
D = 64
NQ = 4                      # q heads per kv group
GCOLS = (NQ + 2) * D        # 384 qkv columns per group
NPAIR = S // 128            # 16 pairs of 64-token blocks
SCALE = 1.0 / 8.0           # 1/sqrt(D)
ROPE_BASE = 10000.0
N_CORES = 8


def _pair_chunks(i):
    """128-token k-chunks feeding query pair i (blocks 2i, 2i+1).

    Diagonal chunk first so its causal mask (GpSimd) overlaps the
    remaining score/PV matmuls instead of gating the PV chain tail.
    """
    chunks = [i]
    if i >= 12:
        chunks.append(3)
    chunks += list(range(max(0, i - 8), i))
    return chunks


def _duos(chunks):
    return [tuple(chunks[j:j + 2]) for j in range(0, len(chunks), 2)]


class _Fills:
    """Round-robin emitter for filler tensor work inside a window."""

    def __init__(self, items, slots):
        self.items = list(items)
        self.slots = max(slots, 1)

    def emit(self):
        if not self.items:
            return
        n = (len(self.items) + self.slots - 1) // self.slots
        for _ in range(min(n, len(self.items))):
            self.items.pop(0)()
        self.slots -= 1

    def drain(self):
        for it in self.items:
            it()
        self.items = []


def _build_nc():
    nc = bacc.Bacc()

    ht = nc.declare_dram_parameter("ht", [HID, S], F16, isOutput=False)
    wq = nc.declare_dram_parameter("wq", [128, 16 * GCOLS], F16, isOutput=False)
    bq = nc.declare_dram_parameter("bq", [128, 3], F32, isOutput=False)
    wd = nc.declare_dram_parameter("wd", [128, 2 * HID], F16, isOutput=False)
    cosq = nc.declare_dram_parameter("cosq", [128, S], F16, isOutput=False)
    sinq = nc.declare_dram_parameter("sinq", [128, S], F16, isOutput=False)
    cosk = nc.declare_dram_parameter("cosk", [64, S], F16, isOutput=False)
    sink = nc.declare_dram_parameter("sink", [64, S], F16, isOutput=False)
    tri = nc.declare_dram_parameter("tri", [128, 128], F16, isOutput=False)
    out = nc.declare_dram_parameter("out", [S, HID], F16, isOutput=True)

    with tile.TileContext(nc) as tc, ExitStack() as ctx:
        consts = ctx.enter_context(tc.tile_pool(name="consts", bufs=1))
        persist = ctx.enter_context(tc.tile_pool(name="persist", bufs=1))

        wq_sb = consts.tile([128, 16 * GCOLS], F16)
        wd_sb = consts.tile([128, 2 * HID], F16)
        bq_sb = consts.tile([128, 3], F32)
        cosq_sb = consts.tile([128, S], F16)
        sinq_sb = consts.tile([128, S], F16)
        cosk_sb = consts.tile([64, S], F16)
        sink_sb = consts.tile([64, S], F16)
        tri_sb = consts.tile([128, 128], F16)
        expb = consts.tile([128, 1], F32)
        nc.vector.memset(expb[:], -5.0)

        # persistent activations
        qkv = [persist.tile([128, S], F16, tag=f"qkv{m}", name=f"qkv{m}")
               for m in range(3)]
        qS2 = persist.tile([128, NQ * S], F16)   # [dup(d), pair*512 + h*128 + t]
        kT2 = persist.tile([128, S], F16)        # [dup(d), t]
        v_sb = persist.tile([128, 16 * 128], F16)  # [t, chunk*128 + (ones | d)]
        ctx_sb = persist.tile([128, 2 * S], F16)   # [(h%2)*64+d, (h//2)*2048+t]

        # ones in cols 0:64 of each chunk: PV emits Z at PSUM rows 0:64
        # (base partition 0 — reciprocal_approx_fast misreads at base 64)
        v_r = v_sb[:].rearrange("p (c w) -> p c w", w=128)
        nc.vector.memset(v_r[:, :, 0:64], 1.0)

        # ---- input DMA: hidden stream + weights ----
        # ht quarters on sync HWDGE; weights/consts on scalar HWDGE (ACT is
        # idle early); qS2/kT2 dup + nothing else on gpsimd SWDGE.
        def load_ht_slice(n):
            tiles = []
            for q in range(4):
                hq = hp.tile([128, 4 * 512], F16, tag="h", name=f"h{n}_{q}")
                src = ht[q * 512:(q + 1) * 512,
                         n * 512:(n + 1) * 512].rearrange(
                    "(c p) t -> p c t", p=128)
                nc.sync.dma_start(
                    out=hq[:].rearrange("p (c t) -> p c t", c=4), in_=src)
                tiles.append(hq)
            return tiles

        hp = ctx.enter_context(tc.tile_pool(name="hp", bufs=8))
        rp = ctx.enter_context(tc.tile_pool(name="rope", bufs=2))
        exp_p = ctx.enter_context(tc.tile_pool(name="exp", bufs=4))
        rec_p = ctx.enter_context(tc.tile_pool(name="rec", bufs=2))
        stg_p = ctx.enter_context(tc.tile_pool(name="stg", bufs=2))
        duo_p = ctx.enter_context(
            tc.tile_pool(name="duo", bufs=1, space="PSUM"))
        psc = ctx.enter_context(tc.tile_pool(name="psc", bufs=2, space="PSUM"))

        # prologue: first hidden quarters + first wq chunk before the rest
        nc.scalar.dma_start(out=wq_sb[:, 0:GCOLS], in_=wq[:, 0:GCOLS])
        h_tiles = {0: load_ht_slice(0)}
        for j in range(3):
            lo, hi = (1 + 5 * j) * GCOLS, (6 + 5 * j) * GCOLS
            nc.scalar.dma_start(out=wq_sb[:, lo:hi], in_=wq[:, lo:hi])
        nc.scalar.dma_start(out=bq_sb[:], in_=bq[:, :])
        nc.scalar.dma_start(out=tri_sb[:], in_=tri[:, :])
        for t_, src in ((cosq_sb, cosq), (sinq_sb, sinq),
                        (cosk_sb, cosk), (sink_sb, sink)):
            nc.scalar.dma_start(out=t_[:], in_=src[:, :])

        acc_sl = {}

        def qkv_triple(n, kc):
            for mc in range(3):
                nc.tensor.matmul(
                    acc_sl[n][mc][:],
                    wq_sb[:, kc * GCOLS + mc * 128: kc * GCOLS + (mc + 1) * 128],
                    h_tiles[n][kc // 4][:, (kc % 4) * 512:(kc % 4 + 1) * 512],
                    start=(kc == 0), stop=(kc == 15))

        def boundary(n):
            """bias-add + rope + v-transpose + dup for finished slice n."""
            nsl = slice(n * 512, (n + 1) * 512)
            for mc in range(3):
                nc.vector.tensor_scalar_add(
                    qkv[mc][:, nsl], acc_sl[n][mc][:], bq_sb[:, mc:mc + 1])
            # v transposes (XBAR DMA) as soon as qkv[2] rows 64:128 exist
            for cc in range(4):
                c = 4 * n + cc
                nc.sync.dma_start_transpose(
                    out=v_sb[:, c * 128 + 64:(c + 1) * 128],
                    in_=qkv[2][64:128, c * 128:(c + 1) * 128])
            # rope on q heads
            for ti in range(2):
                qt = qkv[ti]
                rot = rp.tile([128, 512], F16, tag="rot", name="rot")
                for blk in range(4):
                    src = (blk ^ 1) * 32
                    nc.vector.tensor_copy(rot[blk * 32:(blk + 1) * 32, :],
                                          qt[src:src + 32, nsl])
                tmp = rp.tile([128, 512], F16, tag="tmp", name="tmp")
                nc.vector.tensor_mul(tmp[:], qt[:, nsl], cosq_sb[:, nsl])
                nc.vector.tensor_mul(rot[:], rot[:], sinq_sb[:, nsl])
                for half in range(2):  # head 2*ti + half
                    h = 2 * ti + half
                    dst = qS2[0:64, n * 2048:(n + 1) * 2048].rearrange(
                        "p (pp hh t) -> p pp hh t", hh=NQ, t=128)[:, :, h, :]
                    nc.vector.tensor_add(
                        dst,
                        tmp[half * 64:(half + 1) * 64, :].rearrange(
                            "p (pp t) -> p pp t", t=128),
                        rot[half * 64:(half + 1) * 64, :].rearrange(
                            "p (pp t) -> p pp t", t=128))
            # k rope (qkv[2] rows 0:64)
            rotk = rp.tile([128, 512], F16, tag="rot", name="rotk")
            nc.vector.tensor_copy(rotk[0:32, :], qkv[2][32:64, nsl])
            nc.vector.tensor_copy(rotk[32:64, :], qkv[2][0:32, nsl])
            tmpk = rp.tile([128, 512], F16, tag="tmp", name="tmpk")
            nc.vector.tensor_mul(tmpk[0:64, :], qkv[2][0:64, nsl],
                                 cosk_sb[:, nsl])
            nc.vector.tensor_mul(rotk[0:64, :], rotk[0:64, :], sink_sb[:, nsl])
            nc.vector.tensor_add(kT2[0:64, nsl], tmpk[0:64, :], rotk[0:64, :])
            # replicate to partitions 64:127 for row-tiled score matmuls
            qsl = slice(n * 2048, (n + 1) * 2048)
            nc.gpsimd.dma_start(out=qS2[64:128, qsl], in_=qS2[0:64, qsl])
            nc.gpsimd.dma_start(out=kT2[64:128, nsl], in_=kT2[0:64, nsl])

        tb = tri_sb[:]
        tri_b = bass.AP(tensor=tb.tensor, offset=tb.offset,
                        ap=[tb.ap[0], [0, NQ]] + list(tb.ap[1:]))

        def mask_ex(i, c, exs):
            """block-sparsity masks on a [128,512] exp'd chunk (GpSimd)."""
            if c == i:  # diagonal: causal mask, tri broadcast over 4 heads
                exr = exs.rearrange("p (hh t) -> p hh t", hh=NQ)
                nc.gpsimd.tensor_mul(exr, exr, tri_b)
            elif i >= 8 and c == i - 8:
                nc.gpsimd.memset(exs[0:64, :], 0.0)
                if i % 4 != 3:
                    exr = exs[64:128, :].rearrange(
                        "p (hh t) -> p hh t", hh=NQ)
                    nc.gpsimd.memset(exr[:, :, 64:128], 0.0)
            elif i >= 12 and c == 3:
                nc.gpsimd.memset(exs[0:64, :], 0.0)

        def emit_pair(i, fills):
            chunks = _pair_chunks(i)
            duos = _duos(chunks)
            ctx_ps = psc.tile([128, 512], F32, name="ctx_ps")
            pv_cnt = [0]
            n_pv = len(chunks)

            def pv(ex, duo):
                for s, c in enumerate(duo):
                    nc.tensor.matmul(ctx_ps[:], v_sb[:, c * 128:(c + 1) * 128],
                                     ex[:, s * 512:(s + 1) * 512],
                                     start=(pv_cnt[0] == 0),
                                     stop=(pv_cnt[0] == n_pv - 1))
                    pv_cnt[0] += 1

            prev = None
            for duo in duos:
                sps = duo_p.tile([128, 1024], F32, tag="sps", name="sps")
                for s, c in enumerate(duo):
                    half = slice(s * 64, s * 64 + 64)
                    nc.tensor.matmul(
                        sps[:, s * 512:(s + 1) * 512],
                        kT2[half, c * 128:(c + 1) * 128],
                        qS2[half, i * 512:(i + 1) * 512],
                        start=True, stop=True)
                ex = exp_p.tile([128, 1024], F16, tag="ex", name="ex")
                w = 512 * len(duo)
                nc.scalar.activation(ex[:, 0:w], sps[:, 0:w], AF.Exp,
                                     bias=expb[:])
                for s, c in enumerate(duo):
                    mask_ex(i, c, ex[:, s * 512:(s + 1) * 512])
                if prev is not None:
                    pv(*prev)
                fills.emit()
                prev = (ex, duo)
            pv(*prev)
            # normalize: rows 0:63 of ctx_ps hold the denominators
            rec = rec_p.tile([64, 512], F32, tag="rec", name="rec")
            nc.vector.reciprocal_approx_fast(rec[:], ctx_ps[0:64, :])
            for h in range(NQ):
                nc.vector.tensor_mul(
                    ctx_sb[(h % 2) * 64:(h % 2) * 64 + 64,
                           (h // 2) * S + i * 128:(h // 2) * S + (i + 1) * 128],
                    ctx_ps[64:128, h * 128:(h + 1) * 128],
                    rec[:, h * 128:(h + 1) * 128])

        stg_of = {}

        def dense_unit(i, nn, pool):
            if nn == 0:
                stg_of[i] = stg_p.tile([128, HID], F16, tag="stg", name="stg")
            dps = pool.tile([128, 512], F32, tag="dps", name="dps")
            nc.tensor.matmul(dps[:], ctx_sb[:, i * 128:(i + 1) * 128],
                             wd_sb[:, nn * 512:(nn + 1) * 512],
                             start=True, stop=False)
            nc.tensor.matmul(dps[:],
                             ctx_sb[:, S + i * 128: S + (i + 1) * 128],
                             wd_sb[:, HID + nn * 512: HID + (nn + 1) * 512],
                             start=False, stop=True)
            if nn % 2 == 0:
                nc.scalar.copy(stg_of[i][:, nn * 512:(nn + 1) * 512], dps[:])
            else:
                nc.vector.tensor_copy(stg_of[i][:, nn * 512:(nn + 1) * 512],
                                      dps[:])
            if nn == 3:
                nc.sync.dma_start(out=out[i * 128:(i + 1) * 128, :],
                                  in_=stg_of.pop(i)[:])

        # ================= fused schedule =================
        def triples(n):
            return [(lambda n_, kc_: (lambda: qkv_triple(n_, kc_)))(n, kc)
                    for kc in range(16)]

        def dense_items(pairs, pool):
            return [(lambda i_, nn_: (lambda: dense_unit(i_, nn_, pool)))(i, nn)
                    for i in pairs for nn in range(4)]

        def window(pairs, items, prime):
            slots = sum(len(_duos(_pair_chunks(i))) for i in pairs)
            fills = _Fills(items, slots)
            for _ in range(prime):
                if fills.items:
                    fills.items.pop(0)()
            for i in pairs:
                emit_pair(i, fills)
            fills.drain()

        psd1 = ctx.enter_context(
            tc.tile_pool(name="psd1", bufs=1, space="PSUM"))
        with tc.tile_pool(name="acc", bufs=1, space="PSUM") as accp:
            # W0/W1: slices 0 and 1, plain
            for n in (0, 1):
                acc_sl[n] = [accp.tile([128, 512], F32, tag=f"a{m}",
                                       name=f"acc{n}{m}") for m in range(3)]
                if n == 1:
                    h_tiles[1] = load_ht_slice(1)
                    nc.scalar.dma_start(out=wd_sb[:, 0:HID], in_=wd[:, 0:HID])
                    nc.scalar.dma_start(out=wd_sb[:, HID:2 * HID],
                                        in_=wd[:, HID:2 * HID])
                for kc in range(16):
                    qkv_triple(n, kc)
                boundary(n)

            # W2: pairs 0-3 + slice 2 (boundary rides as a fill so rope/dup
            # complete well before the next pair group needs them)
            acc_sl[2] = [accp.tile([128, 512], F32, tag=f"a{m}",
                                   name=f"acc2{m}") for m in range(3)]
            h_tiles[2] = load_ht_slice(2)
            h_tiles[3] = load_ht_slice(3)
            window((0, 1, 2, 3), triples(2) + [lambda: boundary(2)], prime=2)

            # W3: pairs 4-7 + slice 3 + dense 0-2
            acc_sl[3] = [accp.tile([128, 512], F32, tag=f"a{m}",
                                   name=f"acc3{m}") for m in range(3)]
            window((4, 5, 6, 7),
                   triples(3) + [lambda: boundary(3)]
                   + dense_items(range(0, 3), psd1), prime=2)
        # acc pool closed: 3 PSUM banks back -> dense can double-buffer
        with tc.tile_pool(name="psd2", bufs=2, space="PSUM") as psd2:
            window((8, 9, 10, 11), dense_items(range(3, 9), psd2), prime=4)
            window((12, 13, 14, 15), dense_items(range(9, 14), psd2), prime=4)
            for i in (14, 15):
                for nn in range(4):
                    dense_unit(i, nn, psd2)

    nc.finalize()
    return nc


_NC_CACHE = {}


def _get_nc():
    if "nc" not in _NC_CACHE:
        _NC_CACHE["nc"] = _build_nc()
    return _NC_CACHE["nc"]


def _host_inputs(hidden_states, w_qkv, b_qkv, w_dense):
    h = np.asarray(hidden_states, dtype=np.float32).reshape(S, HID)
    w_qkv = np.asarray(w_qkv, dtype=np.float32)
    b_qkv = np.asarray(b_qkv, dtype=np.float32)
    w_dense = np.asarray(w_dense, dtype=np.float32)

    ht = np.ascontiguousarray(h.T).astype(np.float16)

    inv = 1.0 / (ROPE_BASE ** (np.arange(0, D, 2, dtype=np.float32) / D))
    ang = np.arange(S, dtype=np.float32)[:, None] * inv[None, :]   # [S, 32]
    cosT = np.ascontiguousarray(np.cos(ang).T.astype(np.float32))  # [32, S]
    sinT = np.ascontiguousarray(np.sin(ang).T.astype(np.float32))
    cosq = (np.tile(cosT, (4, 1)) * SCALE).astype(np.float16)
    sinq = (np.concatenate([-sinT, sinT, -sinT, sinT], 0) * SCALE).astype(np.float16)
    cosk = np.tile(cosT, (2, 1)).astype(np.float16)
    sink = np.concatenate([-sinT, sinT], 0).astype(np.float16)

    tri = np.triu(np.ones((128, 128), np.float16))

    in_maps = []
    for g in range(N_CORES):
        wqg = w_qkv[g * GCOLS:(g + 1) * GCOLS, :].T          # [HID, 384]
        wq_t = np.ascontiguousarray(
            wqg.reshape(16, 128, GCOLS).transpose(1, 0, 2).reshape(128, 16 * GCOLS)).astype(np.float16)
        bqg = np.ascontiguousarray(
            b_qkv[g * GCOLS:(g + 1) * GCOLS].reshape(3, 128).T)
        wdg = w_dense[:, g * NQ * D:(g + 1) * NQ * D].T      # [256, HID]
        wd_t = np.ascontiguousarray(
            wdg.reshape(2, 128, HID).transpose(1, 0, 2).reshape(128, 2 * HID)).astype(np.float16)
        in_maps.append({
            "ht": ht, "wq": wq_t, "bq": bqg, "wd": wd_t,
            "cosq": np.ascontiguousarray(cosq), "sinq": np.ascontiguousarray(sinq),
            "cosk": np.ascontiguousarray(cosk), "sink": np.ascontiguousarray(sink),
            "tri": tri,
        })
    return in_maps


def run_device(hidden_states, w_qkv, b_qkv, w_dense, **run_kwargs):
    nc = _get_nc()
    in_maps = _host_inputs(hidden_states, w_qkv, b_qkv, w_dense)
    return run_bass_kernel_spmd(nc, in_maps, list(range(N_CORES)), **run_kwargs)


def kernel(hidden_states, w_qkv, b_qkv, w_dense, b_dense):
    res = run_device(hidden_states, w_qkv, b_qkv, w_dense)
    acc = np.zeros((S, HID), dtype=np.float32)
    for r in res.results:
        acc += r["out"].astype(np.float32)
    acc += np.asarray(b_dense, dtype=np.float32)[None, :]
    return acc.reshape(1, S, HID)


# revision 11
# speedup vs baseline: 1.2578x; 1.2578x over previous
"""TLGv4 block-sparse self-attention on 8 trn2 NeuronCores.

Sharding: tensor-parallel over the 8 KV groups (1 group = 4 Q heads + 1 K +
1 V head per core). Each core computes its group's QKV projection columns,
RoPE, block-sparse attention for its 4 Q heads, and a row-sharded partial of
the dense output projection (f16). Host sums the 8 partials (+ b_dense).

Two phases, each tuned to keep the PE stream homogeneous (216ns/matmul):
  1) QKV: wq_g @ hidden^T accumulated in 3 double-buffered PSUM banks per
     512-token slice; bias-add + RoPE on DVE produce qS2/kT2 with rows
     replicated to partitions 64:127 (SBUF->SBUF DMA on the scalar queue)
     so score matmuls for TWO k-chunks run as concurrent PE row-group
     tiles (K=64 each). v^T via XBAR transpose-DMA; 64 ones-columns per
     chunk make the PV matmul emit softmax denominators as ctx rows 0:63.
  2) Attention + dense: per pair, score duos -> one merged [128,1024] Exp
     on ACT (2 PSUM banks) -> block-sparsity masks on GpSimd (diagonal
     chunk first so masks never gate the PV tail) -> PV chain; 1/Z via
     DVE reciprocal_approx_fast from PSUM; 2 merged normalize muls; dense
     trails 3 pairs behind, PSUM -> f16 staging (copies split DVE/ACT) ->
     one 512KB store per pair.
"""
import numpy as np
from contextlib import ExitStack

import concourse.bacc as bacc
import concourse.bass as bass
import concourse.mybir as mybir
import concourse.tile as tile
from concourse.bass_utils import run_bass_kernel_spmd

F32 = mybir.dt.float32
F16 = mybir.dt.float16
AF = mybir.ActivationFunctionType

S = 2048
HID = 2048
D = 64
NQ = 4                      # q heads per kv group
GCOLS = (NQ + 2) * D        # 384 qkv columns per group
NPAIR = S // 128            # 16 pairs of 64-token blocks
SCALE = 1.0 / 8.0           # 1/sqrt(D)
ROPE_BASE = 10000.0
N_CORES = 8


def _pair_chunks(i):
    """128-token k-chunks feeding query pair i (blocks 2i, 2i+1).

    Diagonal chunk first so its causal mask (GpSimd) overlaps the
    remaining score/PV matmuls instead of gating the PV chain tail.
    """
    chunks = [i]
    if i >= 12:
        chunks.append(3)
    chunks += list(range(max(0, i - 8), i))
    return chunks


def _duos(chunks):
    return [tuple(chunks[j:j + 2]) for j in range(0, len(chunks), 2)]


def _build_nc():
    nc = bacc.Bacc()

    ht = nc.declare_dram_parameter("ht", [HID, S], F16, isOutput=False)
    wq = nc.declare_dram_parameter("wq", [128, 16 * GCOLS], F16, isOutput=False)
    bq = nc.declare_dram_parameter("bq", [128, 3], F32, isOutput=False)
    wd = nc.declare_dram_parameter("wd", [128, 2 * HID], F16, isOutput=False)
    cosq = nc.declare_dram_parameter("cosq", [128, S], F16, isOutput=False)
    sinq = nc.declare_dram_parameter("sinq", [128, S], F16, isOutput=False)
    cosk = nc.declare_dram_parameter("cosk", [64, S], F16, isOutput=False)
    sink = nc.declare_dram_parameter("sink", [64, S], F16, isOutput=False)
    tri = nc.declare_dram_parameter("tri", [128, 128], F16, isOutput=False)
    out = nc.declare_dram_parameter("out", [S, HID], F16, isOutput=True)

    with tile.TileContext(nc) as tc, ExitStack() as ctx:
        consts = ctx.enter_context(tc.tile_pool(name="consts", bufs=1))
        persist = ctx.enter_context(tc.tile_pool(name="persist", bufs=1))

        wq_sb = consts.tile([128, 16 * GCOLS], F16)
        wd_sb = consts.tile([128, 2 * HID], F16)
        bq_sb = consts.tile([128, 3], F32)
        cosq_sb = consts.tile([128, S], F16)
        sinq_sb = consts.tile([128, S], F16)
        cosk_sb = consts.tile([64, S], F16)
        sink_sb = consts.tile([64, S], F16)
        tri_sb = consts.tile([128, 128], F16)
        expb = consts.tile([128, 1], F32)
        nc.vector.memset(expb[:], -5.0)

        # persistent activations
        qkv = [persist.tile([128, S], F16, tag=f"qkv{m}", name=f"qkv{m}")
               for m in range(3)]
        qS2 = persist.tile([128, NQ * S], F16)   # [dup(d), pair*512 + h*128 + t]
        kT2 = persist.tile([128, S], F16)        # [dup(d), t]
        v_sb = persist.tile([128, 16 * 128], F16)  # [t, chunk*128 + (ones | d)]
        ctx_sb = persist.tile([128, 2 * S], F16)   # [(h%2)*64+d, (h//2)*2048+t]

        # ones in cols 0:64 of each chunk: PV emits Z at PSUM rows 0:63
        # (base partition 0 — reciprocal_approx_fast misreads at base 64)
        v_r = v_sb[:].rearrange("p (c w) -> p c w", w=128)
        nc.vector.memset(v_r[:, :, 0:64], 1.0)

        hp = ctx.enter_context(tc.tile_pool(name="hp", bufs=8))
        rp = ctx.enter_context(tc.tile_pool(name="rope", bufs=2))
        exp_p = ctx.enter_context(tc.tile_pool(name="exp", bufs=4))
        rec_p = ctx.enter_context(tc.tile_pool(name="rec", bufs=2))
        stg_p = ctx.enter_context(tc.tile_pool(name="stg", bufs=2))

        def load_ht_slice(n):
            tiles = []
            for q in range(4):
                hq = hp.tile([128, 4 * 512], F16, tag="h", name=f"h{n}_{q}")
                src = ht[q * 512:(q + 1) * 512,
                         n * 512:(n + 1) * 512].rearrange(
                    "(c p) t -> p c t", p=128)
                nc.sync.dma_start(
                    out=hq[:].rearrange("p (c t) -> p c t", c=4), in_=src)
                tiles.append(hq)
            return tiles

        # prologue: hidden stream (sync queue) first, weights on scalar queue
        h_tiles = {0: load_ht_slice(0)}
        nc.scalar.dma_start(out=wq_sb[:, 0:GCOLS], in_=wq[:, 0:GCOLS])
        for j in range(3):
            lo, hi = (1 + 5 * j) * GCOLS, (6 + 5 * j) * GCOLS
            nc.scalar.dma_start(out=wq_sb[:, lo:hi], in_=wq[:, lo:hi])
        nc.scalar.dma_start(out=bq_sb[:], in_=bq[:, :])
        nc.scalar.dma_start(out=tri_sb[:], in_=tri[:, :])

        def boundary(n, acc):
            """bias-add + rope + v-transpose + dup for finished slice n."""
            nsl = slice(n * 512, (n + 1) * 512)
            for mc in range(3):
                nc.vector.tensor_scalar_add(
                    qkv[mc][:, nsl], acc[mc][:], bq_sb[:, mc:mc + 1])
            for cc in range(4):
                c = 4 * n + cc
                nc.sync.dma_start_transpose(
                    out=v_sb[:, c * 128 + 64:(c + 1) * 128],
                    in_=qkv[2][64:128, c * 128:(c + 1) * 128])
            for ti in range(2):
                qt = qkv[ti]
                rot = rp.tile([128, 512], F16, tag="rot", name="rot")
                for blk in range(4):
                    src = (blk ^ 1) * 32
                    nc.vector.tensor_copy(rot[blk * 32:(blk + 1) * 32, :],
                                          qt[src:src + 32, nsl])
                tmp = rp.tile([128, 512], F16, tag="tmp", name="tmp")
                nc.vector.tensor_mul(tmp[:], qt[:, nsl], cosq_sb[:, nsl])
                nc.vector.tensor_mul(rot[:], rot[:], sinq_sb[:, nsl])
                for half in range(2):  # head 2*ti + half
                    h = 2 * ti + half
                    dst = qS2[0:64, n * 2048:(n + 1) * 2048].rearrange(
                        "p (pp hh t) -> p pp hh t", hh=NQ, t=128)[:, :, h, :]
                    nc.vector.tensor_add(
                        dst,
                        tmp[half * 64:(half + 1) * 64, :].rearrange(
                            "p (pp t) -> p pp t", t=128),
                        rot[half * 64:(half + 1) * 64, :].rearrange(
                            "p (pp t) -> p pp t", t=128))
            rotk = rp.tile([128, 512], F16, tag="rot", name="rotk")
            nc.vector.tensor_copy(rotk[0:32, :], qkv[2][32:64, nsl])
            nc.vector.tensor_copy(rotk[32:64, :], qkv[2][0:32, nsl])
            tmpk = rp.tile([128, 512], F16, tag="tmp", name="tmpk")
            nc.vector.tensor_mul(tmpk[0:64, :], qkv[2][0:64, nsl],
                                 cosk_sb[:, nsl])
            nc.vector.tensor_mul(rotk[0:64, :], rotk[0:64, :], sink_sb[:, nsl])
            nc.vector.tensor_add(kT2[0:64, nsl], tmpk[0:64, :], rotk[0:64, :])
            # replicate to partitions 64:127 for row-tiled score matmuls
            qsl = slice(n * 2048, (n + 1) * 2048)
            nc.scalar.dma_start(out=qS2[64:128, qsl], in_=qS2[0:64, qsl])
            nc.scalar.dma_start(out=kT2[64:128, nsl], in_=kT2[0:64, nsl])

        # ---- phase 1: QKV projection (homogeneous PE stream) ----
        with tc.tile_pool(name="psq", bufs=2, space="PSUM") as psq:
            for n in range(4):
                acc = [psq.tile([128, 512], F32, tag=f"a{m}", name=f"acc{m}")
                       for m in range(3)]
                if n == 0:
                    pass
                elif n < 3:
                    h_tiles[n] = h_tiles.get(n) or load_ht_slice(n)
                for kc in range(16):
                    for mc in range(3):
                        nc.tensor.matmul(
                            acc[mc][:],
                            wq_sb[:, kc * GCOLS + mc * 128:
                                  kc * GCOLS + (mc + 1) * 128],
                            h_tiles[n][kc // 4][:, (kc % 4) * 512:
                                                (kc % 4 + 1) * 512],
                            start=(kc == 0), stop=(kc == 15))
                if n == 0:
                    # consts after slice-0 compute is dispatched: they ride
                    # behind the hidden stream instead of starving it
                    for t_, src_ in ((cosq_sb, cosq), (sinq_sb, sinq),
                                     (cosk_sb, cosk), (sink_sb, sink)):
                        nc.scalar.dma_start(out=t_[:], in_=src_[:, :])
                    h_tiles[1] = load_ht_slice(1)
                if n == 2:
                    h_tiles[3] = load_ht_slice(3)
                    nc.scalar.dma_start(out=wd_sb[:, 0:HID], in_=wd[:, 0:HID])
                    nc.scalar.dma_start(out=wd_sb[:, HID:2 * HID],
                                        in_=wd[:, HID:2 * HID])
                boundary(n, acc)

        # ---- phase 2: attention pairs + trailing dense ----
        tb = tri_sb[:]
        tri_b = bass.AP(tensor=tb.tensor, offset=tb.offset,
                        ap=[tb.ap[0], [0, NQ]] + list(tb.ap[1:]))

        def mask_ex(i, c, exs):
            """block-sparsity masks on a [128,512] exp'd chunk (GpSimd)."""
            if c == i:  # diagonal: causal mask, tri broadcast over 4 heads
                exr = exs.rearrange("p (hh t) -> p hh t", hh=NQ)
                nc.gpsimd.tensor_mul(exr, exr, tri_b)
            elif i >= 8 and c == i - 8:
                nc.gpsimd.memset(exs[0:64, :], 0.0)
                if i % 4 != 3:
                    exr = exs[64:128, :].rearrange(
                        "p (hh t) -> p hh t", hh=NQ)
                    nc.gpsimd.memset(exr[:, :, 64:128], 0.0)
            elif i >= 12 and c == 3:
                nc.gpsimd.memset(exs[0:64, :], 0.0)

        with tc.tile_pool(name="duo", bufs=2, space="PSUM") as duo_p, \
             tc.tile_pool(name="psc", bufs=2, space="PSUM") as psc, \
             tc.tile_pool(name="psd", bufs=2, space="PSUM") as psd:

            def emit_pair(i):
                chunks = _pair_chunks(i)
                duos = _duos(chunks)
                ctx_ps = psc.tile([128, 512], F32, name="ctx_ps")
                pv_cnt = [0]
                n_pv = len(chunks)

                def pv(ex, duo):
                    for s, c in enumerate(duo):
                        nc.tensor.matmul(ctx_ps[:],
                                         v_sb[:, c * 128:(c + 1) * 128],
                                         ex[:, s * 512:(s + 1) * 512],
                                         start=(pv_cnt[0] == 0),
                                         stop=(pv_cnt[0] == n_pv - 1))
                        pv_cnt[0] += 1

                prev = None
                for duo in duos:
                    sps = duo_p.tile([128, 1024], F32, tag="sps", name="sps")
                    for s, c in enumerate(duo):
                        half = slice(s * 64, s * 64 + 64)
                        nc.tensor.matmul(
                            sps[:, s * 512:(s + 1) * 512],
                            kT2[half, c * 128:(c + 1) * 128],
                            qS2[half, i * 512:(i + 1) * 512],
                            start=True, stop=True)
                    ex = exp_p.tile([128, 1024], F16, tag="ex", name="ex")
                    w = 512 * len(duo)
                    nc.scalar.activation(ex[:, 0:w], sps[:, 0:w], AF.Exp,
                                         bias=expb[:])
                    for s, c in enumerate(duo):
                        mask_ex(i, c, ex[:, s * 512:(s + 1) * 512])
                    if prev is not None:
                        pv(*prev)
                    prev = (ex, duo)
                pv(*prev)
                # normalize: rows 0:63 of ctx_ps hold the denominators;
                # two muls, each covering head pair (h, h+2) via panel APs
                rec = rec_p.tile([64, 512], F32, tag="rec", name="rec")
                nc.vector.reciprocal_approx_fast(rec[:], ctx_ps[0:64, :])
                src = ctx_ps[64:128, :].rearrange("p (hh t) -> p hh t", hh=NQ)
                recr = rec[:].rearrange("p (hh t) -> p hh t", hh=NQ)
                for lo in range(2):  # heads (lo, lo+2)
                    dst = ctx_sb[lo * 64:lo * 64 + 64, :].rearrange(
                        "p (pan t) -> p pan t", pan=2)[:, :, i * 128:(i + 1) * 128]
                    nc.vector.tensor_mul(dst, src[:, lo::2, :],
                                         recr[:, lo::2, :])

            def emit_dense(i):
                stg = stg_p.tile([128, HID], F16, tag="stg", name="stg")
                for nn in range(4):
                    dps = psd.tile([128, 512], F32, tag="dps", name="dps")
                    nc.tensor.matmul(dps[:], ctx_sb[:, i * 128:(i + 1) * 128],
                                     wd_sb[:, nn * 512:(nn + 1) * 512],
                                     start=True, stop=False)
                    nc.tensor.matmul(dps[:],
                                     ctx_sb[:, S + i * 128: S + (i + 1) * 128],
                                     wd_sb[:, HID + nn * 512:
                                           HID + (nn + 1) * 512],
                                     start=False, stop=True)
                    if nn == 1:
                        nc.scalar.copy(stg[:, nn * 512:(nn + 1) * 512], dps[:])
                    else:
                        nc.vector.tensor_copy(
                            stg[:, nn * 512:(nn + 1) * 512], dps[:])
                nc.sync.dma_start(out=out[i * 128:(i + 1) * 128, :],
                                  in_=stg[:])

            for i in range(NPAIR):
                emit_pair(i)
                if i >= 3:
                    emit_dense(i - 3)
            for i in range(NPAIR - 3, NPAIR):
                emit_dense(i)

    nc.finalize()
    return nc


_NC_CACHE = {}


def _get_nc():
    if "nc" not in _NC_CACHE:
        _NC_CACHE["nc"] = _build_nc()
    return _NC_CACHE["nc"]


def _host_inputs(hidden_states, w_qkv, b_qkv, w_dense):
    h = np.asarray(hidden_states, dtype=np.float32).reshape(S, HID)
    w_qkv = np.asarray(w_qkv, dtype=np.float32)
    b_qkv = np.asarray(b_qkv, dtype=np.float32)
    w_dense = np.asarray(w_dense, dtype=np.float32)

    ht = np.ascontiguousarray(h.T).astype(np.float16)

    inv = 1.0 / (ROPE_BASE ** (np.arange(0, D, 2, dtype=np.float32) / D))
    ang = np.arange(S, dtype=np.float32)[:, None] * inv[None, :]   # [S, 32]
    cosT = np.ascontiguousarray(np.cos(ang).T.astype(np.float32))  # [32, S]
    sinT = np.ascontiguousarray(np.sin(ang).T.astype(np.float32))
    cosq = (np.tile(cosT, (4, 1)) * SCALE).astype(np.float16)
    sinq = (np.concatenate([-sinT, sinT, -sinT, sinT], 0) * SCALE).astype(np.float16)
    cosk = np.tile(cosT, (2, 1)).astype(np.float16)
    sink = np.concatenate([-sinT, sinT], 0).astype(np.float16)

    tri = np.triu(np.ones((128, 128), np.float16))

    in_maps = []
    for g in range(N_CORES):
        wqg = w_qkv[g * GCOLS:(g + 1) * GCOLS, :].T          # [HID, 384]
        wq_t = np.ascontiguousarray(
            wqg.reshape(16, 128, GCOLS).transpose(1, 0, 2).reshape(128, 16 * GCOLS)).astype(np.float16)
        bqg = np.ascontiguousarray(
            b_qkv[g * GCOLS:(g + 1) * GCOLS].reshape(3, 128).T)
        wdg = w_dense[:, g * NQ * D:(g + 1) * NQ * D].T      # [256, HID]
        wd_t = np.ascontiguousarray(
            wdg.reshape(2, 128, HID).transpose(1, 0, 2).reshape(128, 2 * HID)).astype(np.float16)
        in_maps.append({
            "ht": ht, "wq": wq_t, "bq": bqg, "wd": wd_t,
            "cosq": np.ascontiguousarray(cosq), "sinq": np.ascontiguousarray(sinq),
            "cosk": np.ascontiguousarray(cosk), "sink": np.ascontiguousarray(sink),
            "tri": tri,
        })
    return in_maps


def run_device(hidden_states, w_qkv, b_qkv, w_dense, **run_kwargs):
    nc = _get_nc()
    in_maps = _host_inputs(hidden_states, w_qkv, b_qkv, w_dense)
    return run_bass_kernel_spmd(nc, in_maps, list(range(N_CORES)), **run_kwargs)


def kernel(hidden_states, w_qkv, b_qkv, w_dense, b_dense):
    res = run_device(hidden_states, w_qkv, b_qkv, w_dense)
    acc = np.zeros((S, HID), dtype=np.float32)
    for r in res.results:
        acc += r["out"].astype(np.float32)
    acc += np.asarray(b_dense, dtype=np.float32)[None, :]
    return acc.reshape(1, S, HID)
